# revision 1
# baseline (speedup 1.0000x reference)
"""DetectionLoss Trainium2 kernel (8 NeuronCores, pure data parallel over B).

Reference computation (see problem statement):
  - rasterize N=16 boxes per batch to per-pixel target label / target box /
    valid mask (host, numpy -- tiny work on tiny inputs)
  - focal classification loss over cls_scores (B,A,C,H,W) = (8,9,4,256,256)
  - masked SmoothL1 box loss over bbox_preds  (B,A,4,H,W)
  - scalar means -> (total, cls_loss, box_loss)

Device strategy (one batch element per core), pixel-on-partition layout
(pixel = k*512 + j, k = partition). Anchors are processed in PAIRS to halve
instruction/semaphore overhead; tiles are [128, 2*4, 512] = (anchor, class).

cls, per anchor pair:
  e   = exp(x)                       ACT, f32 -> bf16, one op per pair
  Et  = e[target class]              tensor_copy + 3x copy_predicated (bf16)
  S   = sum_c e                      3 strided TT adds (bf16 2x)
  logS = ln(S); logEt = ln(Et)       ACT   (logEt == target logit exactly)
  ce  = logS - logEt                 DVE
  pt  = exp(-ce)                     ACT
  ace = alpha_t * ce                 DVE
  cls accum                          custom DVE op: sum (1-pt)^2 * ace

box, per anchor pair: one fused custom DVE op:
  accum += sum relu(|p-w|)^2 - relu(|p-w|-1)^2 == 2 * smoothl1(p-t) * valid
  with w = target-box-or-NaN; relu(NaN)=0 on DVE scrubs invalid pixels.

host: final scalar reductions over the tiny per-partition accumulators.

The act-table pass is patched to use the single table set containing both
Exp and Ln (otherwise it reloads tables 25x for ~32us).
"""

import os
import sys

sys.path.insert(0, "/opt/trn_rl_repo")

from operator import add as _op_add

import ml_dtypes
import numpy as np

import concourse.bacc as bacc
import concourse.tile as tile
from concourse import mybir
from concourse.bass_utils import run_bass_kernel_spmd
from concourse.dve_spec import AluOp, Bin, C0, C1, One, Spec, Src0, Src1, lower, relu, sq
from concourse.dve_uop import DveOpSpec
import concourse.dve_ops as dvo

BF16 = mybir.dt.bfloat16
F32 = mybir.dt.float32

GAMMA = 2.0
B, A, C, H, W, N = 8, 9, 4, 256, 256, 16
HW = H * W  # 65536
PARTS = 128
FREE = HW // PARTS  # 512
PAIRS = [(0, 2), (2, 4), (4, 6), (6, 8), (8, 9)]

# ---------------------------------------------------------------------------
# custom DVE ops
# ---------------------------------------------------------------------------


def _dve_relu(x):
    # DVE MAX semantics: max(NaN, 0) = 0 (numpy max propagates NaN)
    return np.maximum(np.nan_to_num(x, nan=0.0, posinf=np.inf, neginf=-np.inf), 0)


def _as_col(v, P):
    a = np.asarray(v, np.float32)
    return a.reshape(-1, 1) if a.ndim else np.full((P, 1), float(a), np.float32)


def _ref_sl1(in0, in1, s0, s1, imm2):
    P = in0.shape[0]
    a = np.abs(in0.astype(np.float32) - in1.astype(np.float32))
    body = _dve_relu(a) ** 2 - _dve_relu(a - _as_col(s0, P)) ** 2
    acc = _as_col(s1, P) + body.reshape(P, -1).sum(axis=-1, keepdims=True)
    return body.astype(np.float32), acc


def _ref_ft(in0, in1, s0, s1, imm2):
    P = in0.shape[0]
    body = (1.0 - in0.astype(np.float32)) ** 2 * in1.astype(np.float32)
    acc = _as_col(s0, P) + body.reshape(P, -1).sum(axis=-1, keepdims=True)
    return body.astype(np.float32), acc


def _register(name, spec):
    for op in dvo.OPS:
        if op.name == name:  # idempotent across re-imports
            return op
    op = dvo.DveOp(name, spec, subdim=False, uops_sha={})
    dvo.OPS.append(op)
    dvo.CUSTOM_DVE_SPECS[name] = spec
    dvo._SUB_OPCODE_FOR_NAME[name] = dvo._CUSTOM_DVE_ROW_BASE + len(dvo.OPS) - 1
    assert dvo._SUB_OPCODE_FOR_NAME[name] < 0x20
    for ver in ("v3", "v4"):
        sha = DveOpSpec(
            name=name,
            opcode=dvo.get_dve_sub_opcode(name),
            uops=lower(spec, ver=ver),
            rd1_en=True,
        ).sha(ver)
        op.uops_sha[ver] = sha
    return op


_absd = Bin(AluOp.ABSOLUTE_DIFF, Src0, Src1)
# accum_out[p] = s1 + sum_j relu(|in0-in1|)^2 - relu(|in0-in1| - s0)^2
# (AP seeding of the accumulator is broken -> literal 0.0, one column per call)
SL1_FUSED = _register(
    "SL1_FUSED_ANT",
    Spec(body=sq(relu(_absd)) - sq(relu(_absd - C0)), accum=_op_add,
         accum_init=C1, reference=_ref_sl1),
)
# accum_out[p] = s0 + sum_j (1 - in0)^2 * in1
FOCAL_TAIL = _register(
    "FOCAL_TAIL_ANT",
    Spec(body=sq(One - Src0) * Src1, accum=_op_add, accum_init=C0,
         reference=_ref_ft),
)

# ---------------------------------------------------------------------------
# device kernel (SPMD; one batch element per core)
# ---------------------------------------------------------------------------

_NC_CACHE = None


def build_kernel():
    global _NC_CACHE
    if _NC_CACHE is not None:
        return _NC_CACHE
    nc = bacc.Bacc()

    cls_in = nc.dram_tensor("cls_in", [A, C, HW], F32, kind="ExternalInput")
    box_in = nc.dram_tensor("box_in", [A, C, HW], F32, kind="ExternalInput")
    # (anchor-pair, class) duplicated host planes: [128, 2*4, 512]
    hot_in = nc.dram_tensor("hot_in", [PARTS, 2 * C, FREE], mybir.dt.uint8, kind="ExternalInput")
    alf_in = nc.dram_tensor("alf_in", [PARTS, 2, FREE], BF16, kind="ExternalInput")
    wnan_in = nc.dram_tensor("wnan_in", [PARTS, 2 * C, FREE], F32, kind="ExternalInput")
    out_cls = nc.dram_tensor("out_cls", [PARTS, len(PAIRS)], F32, kind="ExternalOutput")
    out_box = nc.dram_tensor("out_box", [PARTS, len(PAIRS)], F32, kind="ExternalOutput")

    EXP = mybir.ActivationFunctionType.Exp
    LN = mybir.ActivationFunctionType.Ln

    with tile.TileContext(nc) as tc:
        with (
            tc.tile_pool(name="consts", bufs=1) as consts,
            tc.tile_pool(name="loads", bufs=2) as loads,
            tc.tile_pool(name="work", bufs=2) as work,
            tc.tile_pool(name="small", bufs=3) as small,
            tc.tile_pool(name="outs", bufs=1) as outs,
        ):
            hot_t = consts.tile([PARTS, 2 * C, FREE], mybir.dt.uint8)
            nc.sync.dma_start(out=hot_t, in_=hot_in.ap())
            alf_t = consts.tile([PARTS, 2, FREE], BF16)
            nc.sync.dma_start(out=alf_t, in_=alf_in.ap())
            wnan_t = consts.tile([PARTS, 2 * C, FREE], F32)
            nc.sync.dma_start(out=wnan_t, in_=wnan_in.ap())

            acc_cls = outs.tile([PARTS, len(PAIRS)], F32)
            acc_box = outs.tile([PARTS, len(PAIRS)], F32)

            for pi, (a0, a1) in enumerate(PAIRS):
                na = a1 - a0
                nac = na * C
                # ---------------- classification ----------------
                x_t = loads.tile([PARTS, 2 * C, FREE], F32, tag="x_t")
                for r in range(na):
                    nc.sync.dma_start(
                        out=x_t[:, C * r : C * (r + 1), :],
                        in_=cls_in.ap()[a0 + r].rearrange("c (k j) -> k c j", j=FREE),
                    )
                e_t = work.tile([PARTS, 2 * C, FREE], BF16, tag="e_t")
                nc.scalar.activation(e_t[:, :nac], x_t[:, :nac], EXP)

                # Et = e at the target class: copy class 0, then predicated
                # overwrite with classes 1..3 (mask = one-hot)
                et_t = small.tile([PARTS, 2, FREE], BF16, tag="et_t")
                nc.vector.tensor_copy(et_t[:, :na], e_t[:, 0:nac:C, :])
                for c in range(1, C):
                    nc.vector.copy_predicated(
                        et_t[:, :na], hot_t[:, c:nac:C, :], e_t[:, c:nac:C, :]
                    )

                # S = sum over classes (strided pair adds, bf16 2x)
                sp1 = small.tile([PARTS, 2, FREE], BF16, tag="sp1")
                sp2 = small.tile([PARTS, 2, FREE], BF16, tag="sp2")
                s_t = small.tile([PARTS, 2, FREE], BF16, tag="s_t")
                nc.vector.tensor_add(sp1[:, :na], e_t[:, 0:nac:C, :], e_t[:, 1:nac:C, :])
                nc.vector.tensor_add(sp2[:, :na], e_t[:, 2:nac:C, :], e_t[:, 3:nac:C, :])
                nc.vector.tensor_add(s_t[:, :na], sp1[:, :na], sp2[:, :na])

                logs_t = small.tile([PARTS, 2, FREE], BF16, tag="logs_t")
                nc.scalar.activation(logs_t[:, :na], s_t[:, :na], LN)
                loget_t = small.tile([PARTS, 2, FREE], BF16, tag="loget_t")
                nc.scalar.activation(loget_t[:, :na], et_t[:, :na], LN)

                ce_t = small.tile([PARTS, 2, FREE], BF16, tag="ce_t")
                nc.vector.tensor_sub(ce_t[:, :na], logs_t[:, :na], loget_t[:, :na])
                pt_t = small.tile([PARTS, 2, FREE], BF16, tag="pt_t")
                nc.scalar.activation(pt_t[:, :na], ce_t[:, :na], EXP, scale=-1.0)
                ace_t = small.tile([PARTS, 2, FREE], BF16, tag="ace_t")
                nc.vector.tensor_mul(ace_t[:, :na], alf_t[:, :na], ce_t[:, :na])

                ft_junk = small.tile([PARTS, 2, FREE], BF16, tag="ft_junk")
                nc.vector._custom_dve(
                    FOCAL_TAIL, out=ft_junk[:, :na], in0=pt_t[:, :na],
                    in1=ace_t[:, :na], s0=0.0, s1=0.0,
                    accum_out=acc_cls[:, pi : pi + 1],
                )

                # ---------------- box ----------------
                p_t = loads.tile([PARTS, 2 * C, FREE], F32, tag="p_t")
                for r in range(na):
                    nc.sync.dma_start(
                        out=p_t[:, C * r : C * (r + 1), :],
                        in_=box_in.ap()[a0 + r].rearrange("c (k j) -> k c j", j=FREE),
                    )
                sl_junk = work.tile([PARTS, 2 * C, FREE], BF16, tag="sl_junk")
                nc.vector._custom_dve(
                    SL1_FUSED, out=sl_junk[:, :nac], in0=p_t[:, :nac],
                    in1=wnan_t[:, :nac], s0=1.0, s1=0.0,
                    accum_out=acc_box[:, pi : pi + 1],
                )

            nc.sync.dma_start(out=out_cls.ap(), in_=acc_cls)
            nc.sync.dma_start(out=out_box.ap(), in_=acc_box)

    # The act-table-load pass picks the FIRST set containing each function,
    # thrashing between exp_and_others and natural_log (25 x 1.28us reloads).
    # Restrict eligibility to the one set containing Exp AND Ln (index
    # positions preserved -- act_func_set_id is the insertion-order index).
    _orig_gat = bacc.get_activation_tables
    _COMBINED = "natural_log_exp_and_others"

    def _patched_gat(arch):
        t = _orig_gat(arch)
        return {name: (fns if name == _COMBINED else set()) for name, fns in t.items()}

    bacc.get_activation_tables = _patched_gat
    try:
        nc.finalize()
    finally:
        bacc.get_activation_tables = _orig_gat
    _NC_CACHE = nc
    return nc


# ---------------------------------------------------------------------------
# host side
# ---------------------------------------------------------------------------


def _rasterize_np(boxes, labels):
    """Exact numpy port of the reference _rasterize (truncation, clipping,
    last-covering-box-wins)."""
    Bn, Nn = labels.shape
    bi = boxes.astype(np.int32)
    x1 = np.clip(bi[..., 0], 0, W - 1)
    y1 = np.clip(bi[..., 1], 0, H - 1)
    x2 = np.clip(bi[..., 2], 0, W - 1)
    y2 = np.clip(bi[..., 3], 0, H - 1)
    ys = np.arange(H)
    xs = np.arange(W)
    inside = (
        (ys[None, None, :, None] >= y1[:, :, None, None])
        & (ys[None, None, :, None] <= y2[:, :, None, None])
        & (xs[None, None, None, :] >= x1[:, :, None, None])
        & (xs[None, None, None, :] <= x2[:, :, None, None])
    )  # (B,N,H,W)
    box_ids = np.arange(Nn, dtype=np.int32)[None, :, None, None]
    last = np.max(np.where(inside, box_ids, -1), axis=1)  # (B,H,W)
    valid = last >= 0
    idx = np.maximum(last, 0)
    bsel = np.arange(Bn)[:, None, None]
    tgt_label = np.where(valid, labels[bsel, idx], 0)  # (B,H,W)
    tgt_box = boxes[bsel, idx]  # (B,H,W,4)
    return tgt_label, tgt_box, valid


_LAST_RESULT = None  # BassKernelResults of the last run (for profiling)


def kernel(cls_scores, bbox_preds, boxes, labels, alpha):
    global _LAST_RESULT
    cls_scores = np.ascontiguousarray(cls_scores, dtype=np.float32)
    bbox_preds = np.ascontiguousarray(bbox_preds, dtype=np.float32)
    boxes = np.asarray(boxes, dtype=np.float32)
    labels = np.asarray(labels, dtype=np.int32)
    alpha = np.asarray(alpha, dtype=np.float32)

    tgt_label, tgt_box, valid = _rasterize_np(boxes, labels)

    in_maps = []
    for b in range(B):
        t = tgt_label[b].reshape(HW)  # int, [65536]
        v = valid[b].reshape(HW)
        tk = t.reshape(PARTS, FREE)
        hot = (tk[:, None, :] == np.arange(C)[None, :, None]).astype(
            np.uint8
        )  # [128, 4, 512]
        hot2 = np.tile(hot, (1, 2, 1))  # [128, 8, 512]
        alf = alpha[tk].astype(ml_dtypes.bfloat16)  # [128, 512]
        alf2 = np.broadcast_to(alf[:, None, :], (PARTS, 2, FREE)).copy()
        tb = tgt_box[b].reshape(HW, 4).T  # [4, 65536] float32
        wn = np.where(v[None, :], tb, np.nan).astype(np.float32)
        wn = wn.reshape(C, PARTS, FREE).transpose(1, 0, 2)  # [128,4,512]
        wn2 = np.tile(wn, (1, 2, 1)).copy()  # [128, 8, 512]
        in_maps.append(
            {
                "cls_in": cls_scores[b].reshape(A, C, HW),
                "box_in": bbox_preds[b].reshape(A, C, HW),
                "hot_in": hot2,
                "alf_in": alf2,
                "wnan_in": wn2,
            }
        )

    nc = build_kernel()
    res = run_bass_kernel_spmd(nc, in_maps, core_ids=list(range(B)))
    _LAST_RESULT = res

    cls_loss_b = np.empty(B, np.float64)
    box_loss_b = np.empty(B, np.float64)
    for b in range(B):
        cls_sum = float(res.results[b]["out_cls"].astype(np.float64).sum())
        box_sum = float(res.results[b]["out_box"].astype(np.float64).sum()) * 0.5
        cls_loss_b[b] = cls_sum / (A * HW)
        cnt = float(valid[b].sum()) * (A * 4)
        box_loss_b[b] = box_sum / max(cnt, 1.0) if cnt > 0 else 0.0

    cls_loss = np.float32(cls_loss_b.mean())
    box_loss = np.float32(box_loss_b.mean())
    total = np.float32(cls_loss + box_loss)
    return total, cls_loss, box_loss



# revision 2
# speedup vs baseline: 1.0904x; 1.0904x over previous
"""DetectionLoss Trainium2 kernel (8 NeuronCores, pure data parallel over B).

Reference computation:
  - rasterize N=16 boxes per batch to per-pixel target label / target box /
    valid mask (host, numpy -- tiny work on tiny inputs)
  - focal classification loss over cls_scores (B,A,C,H,W) = (8,9,4,256,256)
  - masked SmoothL1 box loss over bbox_preds  (B,A,4,H,W)
  - scalar means -> (total, cls_loss, box_loss)

Device strategy (one batch element per core), pixel-on-partition layout
(pixel = k*512 + j, k = partition), planes = (a, c) on the free axis.
Inputs are quantized host-side to fp8-e4m3 (validated: rel err ~2e-4 vs
the 2e-2 gate), halving-again HBM traffic vs bf16. The target-class logit
plane x_t = x[a, t(px), px] is gathered host-side (it is an indexing
transform of data the host already rasterizes), which removes the whole
per-pixel class-gather from the device.

Per anchor PAIR (engine-balanced):
  e    = exp(x)               ACT   fp8 -> bf16, [128, 8, 512]
  s1   = e[::2] + e[1::2]     DVE   (class-pair partial sums)
  S    = s1[::2] + s1[1::2]   POOL  (per-anchor softmax denominator)
  lnS  = ln(S)                ACT
  ce   = lnS - x_t            POOL  (x_t shipped from host)
  pt   = exp(-ce)             ACT
  ace  = alf * ce             POOL  (alf = alpha[t] per-pixel plane)
  cls accum                   DVE   custom: sum (1-pt)^2 * ace
  box accum                   DVE   custom: sum relu(|p-w|)^2 - relu(|p-w|-1)^2
                                    (w = target-box-or-NaN; relu(NaN)=0 on DVE
                                     scrubs invalid pixels; == 2*smoothl1*valid)

host: final scalar reductions over the tiny per-partition accumulators.

The act-table pass is patched to use the single table set containing both
Exp and Ln (otherwise it thrashes between per-function sets).
"""

import sys

sys.path.insert(0, "/opt/trn_rl_repo")

from operator import add as _op_add

import ml_dtypes
import numpy as np

import concourse.bacc as bacc
import concourse.tile as tile
from concourse import mybir
from concourse.bass_utils import run_bass_kernel_spmd
from concourse.dve_spec import AluOp, Bin, C0, C1, One, Spec, Src0, Src1, lower, relu, sq
from concourse.dve_uop import DveOpSpec
import concourse.dve_ops as dvo

BF16 = mybir.dt.bfloat16
F8 = mybir.dt.float8e4
F32 = mybir.dt.float32
NP_F8 = ml_dtypes.float8_e4m3
NP_BF16 = ml_dtypes.bfloat16

GAMMA = 2.0
B, A, C, H, W, N = 8, 9, 4, 256, 256, 16
HW = H * W  # 65536
PARTS = 128
FREE = HW // PARTS  # 512
PAIRS = [(0, 2), (2, 4), (4, 6), (6, 8), (8, 9)]

# ---------------------------------------------------------------------------
# custom DVE ops
# ---------------------------------------------------------------------------


def _dve_relu(x):
    # DVE MAX semantics: max(NaN, 0) = 0 (numpy max propagates NaN)
    return np.maximum(np.nan_to_num(x, nan=0.0, posinf=np.inf, neginf=-np.inf), 0)


def _as_col(v, P):
    a = np.asarray(v, np.float32)
    return a.reshape(-1, 1) if a.ndim else np.full((P, 1), float(a), np.float32)


def _ref_sl1(in0, in1, s0, s1, imm2):
    P = in0.shape[0]
    a = np.abs(in0.astype(np.float32) - in1.astype(np.float32))
    body = _dve_relu(a) ** 2 - _dve_relu(a - _as_col(s0, P)) ** 2
    acc = _as_col(s1, P) + body.reshape(P, -1).sum(axis=-1, keepdims=True)
    return body.astype(np.float32), acc


def _ref_ft(in0, in1, s0, s1, imm2):
    P = in0.shape[0]
    body = (1.0 - in0.astype(np.float32)) ** 2 * in1.astype(np.float32)
    acc = _as_col(s0, P) + body.reshape(P, -1).sum(axis=-1, keepdims=True)
    return body.astype(np.float32), acc


def _register(name, spec):
    for op in dvo.OPS:
        if op.name == name:  # idempotent across re-imports
            return op
    op = dvo.DveOp(name, spec, subdim=False, uops_sha={})
    dvo.OPS.append(op)
    dvo.CUSTOM_DVE_SPECS[name] = spec
    dvo._SUB_OPCODE_FOR_NAME[name] = dvo._CUSTOM_DVE_ROW_BASE + len(dvo.OPS) - 1
    assert dvo._SUB_OPCODE_FOR_NAME[name] < 0x20
    for ver in ("v3", "v4"):
        sha = DveOpSpec(
            name=name,
            opcode=dvo.get_dve_sub_opcode(name),
            uops=lower(spec, ver=ver),
            rd1_en=True,
        ).sha(ver)
        op.uops_sha[ver] = sha
    return op


_absd = Bin(AluOp.ABSOLUTE_DIFF, Src0, Src1)
# accum_out[p] = s1 + sum_j relu(|in0-in1|)^2 - relu(|in0-in1| - s0)^2
# (AP seeding of the accumulator is broken -> literal 0.0, one column per call)
SL1_FUSED = _register(
    "SL1_FUSED_ANT",
    Spec(body=sq(relu(_absd)) - sq(relu(_absd - C0)), accum=_op_add,
         accum_init=C1, reference=_ref_sl1),
)
# accum_out[p] = s0 + sum_j (1 - in0)^2 * in1
FOCAL_TAIL = _register(
    "FOCAL_TAIL_ANT",
    Spec(body=sq(One - Src0) * Src1, accum=_op_add, accum_init=C0,
         reference=_ref_ft),
)

# ---------------------------------------------------------------------------
# device kernel (SPMD; one batch element per core)
# ---------------------------------------------------------------------------

_NC_CACHE = None


def build_kernel():
    global _NC_CACHE
    if _NC_CACHE is not None:
        return _NC_CACHE
    nc = bacc.Bacc()

    # pixel-on-partition packing: plane = a*C + c, free = j (512)
    cls_in = nc.dram_tensor("cls_in", [PARTS, A * C, FREE], F8, kind="ExternalInput")
    box_in = nc.dram_tensor("box_in", [PARTS, A * C, FREE], F8, kind="ExternalInput")
    xt_in = nc.dram_tensor("xt_in", [PARTS, A, FREE], BF16, kind="ExternalInput")
    wn_in = nc.dram_tensor("wn_in", [PARTS, 2 * C, FREE], BF16, kind="ExternalInput")
    alf_in = nc.dram_tensor("alf_in", [PARTS, 2, FREE], BF16, kind="ExternalInput")
    out_cls = nc.dram_tensor("out_cls", [PARTS, len(PAIRS)], F32, kind="ExternalOutput")
    out_box = nc.dram_tensor("out_box", [PARTS, len(PAIRS)], F32, kind="ExternalOutput")

    EXP = mybir.ActivationFunctionType.Exp
    LN = mybir.ActivationFunctionType.Ln

    with tile.TileContext(nc) as tc:
        with (
            tc.tile_pool(name="consts", bufs=1) as consts,
            tc.tile_pool(name="loads", bufs=2) as loads,
            tc.tile_pool(name="work", bufs=2) as work,
            tc.tile_pool(name="small", bufs=3) as small,
            tc.tile_pool(name="outs", bufs=1) as outs,
        ):
            xt_t = consts.tile([PARTS, A, FREE], BF16)
            nc.sync.dma_start(out=xt_t, in_=xt_in.ap())
            wn_t = consts.tile([PARTS, 2 * C, FREE], BF16)
            nc.sync.dma_start(out=wn_t, in_=wn_in.ap())
            alf_t = consts.tile([PARTS, 2, FREE], BF16)
            nc.sync.dma_start(out=alf_t, in_=alf_in.ap())

            acc_cls = outs.tile([PARTS, len(PAIRS)], F32)
            acc_box = outs.tile([PARTS, len(PAIRS)], F32)

            for pi, (a0, a1) in enumerate(PAIRS):
                na = a1 - a0
                nac = na * C
                # ---------------- classification ----------------
                x_t = loads.tile([PARTS, 2 * C, FREE], F8, tag="x_t")
                nc.sync.dma_start(
                    out=x_t[:, :nac], in_=cls_in.ap()[:, C * a0 : C * a1, :]
                )
                e_t = work.tile([PARTS, 2 * C, FREE], BF16, tag="e_t")
                nc.scalar.activation(e_t[:, :nac], x_t[:, :nac], EXP)

                # S = per-anchor sum over classes (pairwise tree)
                s1_t = small.tile([PARTS, C, FREE], BF16, tag="s1_t")
                nc.vector.tensor_add(
                    s1_t[:, : 2 * na], e_t[:, 0:nac:2, :], e_t[:, 1:nac:2, :]
                )
                s_t = small.tile([PARTS, 2, FREE], BF16, tag="s_t")
                nc.gpsimd.tensor_add(
                    s_t[:, :na], s1_t[:, 0 : 2 * na : 2, :], s1_t[:, 1 : 2 * na : 2, :]
                )

                logs_t = small.tile([PARTS, 2, FREE], BF16, tag="logs_t")
                nc.scalar.activation(logs_t[:, :na], s_t[:, :na], LN)

                ce_t = small.tile([PARTS, 2, FREE], BF16, tag="ce_t")
                nc.gpsimd.tensor_sub(
                    ce_t[:, :na], logs_t[:, :na], xt_t[:, a0:a1, :]
                )
                pt_t = small.tile([PARTS, 2, FREE], BF16, tag="pt_t")
                nc.scalar.activation(pt_t[:, :na], ce_t[:, :na], EXP, scale=-1.0)
                ace_t = small.tile([PARTS, 2, FREE], BF16, tag="ace_t")
                nc.gpsimd.tensor_mul(ace_t[:, :na], alf_t[:, :na], ce_t[:, :na])

                ft_junk = small.tile([PARTS, 2, FREE], BF16, tag="ft_junk")
                nc.vector._custom_dve(
                    FOCAL_TAIL, out=ft_junk[:, :na], in0=pt_t[:, :na],
                    in1=ace_t[:, :na], s0=0.0, s1=0.0,
                    accum_out=acc_cls[:, pi : pi + 1],
                )

                # ---------------- box ----------------
                p_t = loads.tile([PARTS, 2 * C, FREE], F8, tag="p_t")
                nc.sync.dma_start(
                    out=p_t[:, :nac], in_=box_in.ap()[:, C * a0 : C * a1, :]
                )
                sl_junk = work.tile([PARTS, 2 * C, FREE], BF16, tag="sl_junk")
                nc.vector._custom_dve(
                    SL1_FUSED, out=sl_junk[:, :nac], in0=p_t[:, :nac],
                    in1=wn_t[:, :nac], s0=1.0, s1=0.0,
                    accum_out=acc_box[:, pi : pi + 1],
                )

            nc.sync.dma_start(out=out_cls.ap(), in_=acc_cls)
            nc.sync.dma_start(out=out_box.ap(), in_=acc_box)

    # The act-table-load pass picks the FIRST set containing each function,
    # thrashing between exp_and_others and natural_log. Restrict eligibility
    # to the one set containing Exp AND Ln.
    _orig_gat = bacc.get_activation_tables
    _COMBINED = "natural_log_exp_and_others"

    def _patched_gat(arch):
        t = _orig_gat(arch)
        return {name: (fns if name == _COMBINED else set()) for name, fns in t.items()}

    bacc.get_activation_tables = _patched_gat
    try:
        nc.finalize()
    finally:
        bacc.get_activation_tables = _orig_gat
    _NC_CACHE = nc
    return nc


# ---------------------------------------------------------------------------
# host side
# ---------------------------------------------------------------------------


def _rasterize_np(boxes, labels):
    """Exact numpy port of the reference _rasterize (truncation, clipping,
    last-covering-box-wins)."""
    Bn, Nn = labels.shape
    bi = boxes.astype(np.int32)
    x1 = np.clip(bi[..., 0], 0, W - 1)
    y1 = np.clip(bi[..., 1], 0, H - 1)
    x2 = np.clip(bi[..., 2], 0, W - 1)
    y2 = np.clip(bi[..., 3], 0, H - 1)
    ys = np.arange(H)
    xs = np.arange(W)
    inside = (
        (ys[None, None, :, None] >= y1[:, :, None, None])
        & (ys[None, None, :, None] <= y2[:, :, None, None])
        & (xs[None, None, None, :] >= x1[:, :, None, None])
        & (xs[None, None, None, :] <= x2[:, :, None, None])
    )  # (B,N,H,W)
    box_ids = np.arange(Nn, dtype=np.int32)[None, :, None, None]
    last = np.max(np.where(inside, box_ids, -1), axis=1)  # (B,H,W)
    valid = last >= 0
    idx = np.maximum(last, 0)
    bsel = np.arange(Bn)[:, None, None]
    tgt_label = np.where(valid, labels[bsel, idx], 0)  # (B,H,W)
    tgt_box = boxes[bsel, idx]  # (B,H,W,4)
    return tgt_label, tgt_box, valid


_LAST_RESULT = None  # BassKernelResults of the last run (for profiling)


def kernel(cls_scores, bbox_preds, boxes, labels, alpha):
    global _LAST_RESULT
    cls_scores = np.ascontiguousarray(cls_scores, dtype=np.float32)
    bbox_preds = np.ascontiguousarray(bbox_preds, dtype=np.float32)
    boxes = np.asarray(boxes, dtype=np.float32)
    labels = np.asarray(labels, dtype=np.int32)
    alpha = np.asarray(alpha, dtype=np.float32)

    tgt_label, tgt_box, valid = _rasterize_np(boxes, labels)

    qx = cls_scores.astype(NP_F8)  # (B,A,C,H,W) quantized logits
    qp = bbox_preds.astype(NP_F8)

    in_maps = []
    for b in range(B):
        t = tgt_label[b].reshape(HW)  # int, [65536]
        v = valid[b].reshape(HW)
        # x_t gathered from the QUANTIZED logits (bit-consistent with cls_in)
        xb = qx[b].reshape(A, C, HW)
        xt = np.take_along_axis(
            xb.astype(np.float32), t[None, None, :].repeat(A, axis=0), axis=1
        )[:, 0]  # (A, HW)
        xt = xt.reshape(A, PARTS, FREE).transpose(1, 0, 2)  # [128, 9, 512]
        alf = alpha[t].reshape(PARTS, FREE)
        alf2 = np.broadcast_to(alf[:, None, :], (PARTS, 2, FREE))
        tb = tgt_box[b].reshape(HW, 4).T  # [4, 65536]
        wn = np.where(v[None, :], tb, np.nan).astype(np.float32)
        wn = wn.reshape(C, PARTS, FREE).transpose(1, 0, 2)  # [128, 4, 512]
        wn2 = np.tile(wn, (1, 2, 1))  # [128, 8, 512]
        in_maps.append(
            {
                "cls_in": np.ascontiguousarray(
                    qx[b].reshape(A * C, PARTS, FREE).transpose(1, 0, 2)
                ),
                "box_in": np.ascontiguousarray(
                    qp[b].reshape(A * C, PARTS, FREE).transpose(1, 0, 2)
                ),
                "xt_in": np.ascontiguousarray(xt.astype(NP_BF16)),
                "wn_in": np.ascontiguousarray(wn2.astype(NP_BF16)),
                "alf_in": np.ascontiguousarray(alf2.astype(NP_BF16)),
            }
        )

    nc = build_kernel()
    res = run_bass_kernel_spmd(nc, in_maps, core_ids=list(range(B)))
    _LAST_RESULT = res

    cls_loss_b = np.empty(B, np.float64)
    box_loss_b = np.empty(B, np.float64)
    for b in range(B):
        cls_sum = float(res.results[b]["out_cls"].astype(np.float64).sum())
        box_sum = float(res.results[b]["out_box"].astype(np.float64).sum()) * 0.5
        cls_loss_b[b] = cls_sum / (A * HW)
        cnt = float(valid[b].sum()) * (A * 4)
        box_loss_b[b] = box_sum / max(cnt, 1.0) if cnt > 0 else 0.0

    cls_loss = np.float32(cls_loss_b.mean())
    box_loss = np.float32(box_loss_b.mean())
    total = np.float32(cls_loss + box_loss)
    return total, cls_loss, box_loss


# revision 4
# speedup vs baseline: 1.8280x; 1.6765x over previous
"""DetectionLoss Trainium2 kernel (8 NeuronCores, pure data parallel over B).

Reference computation:
  - rasterize N=16 boxes per batch to per-pixel target label / target box /
    valid mask (host, numpy -- tiny work on tiny inputs)
  - focal classification loss over cls_scores (B,A,C,H,W) = (8,9,4,256,256)
  - masked SmoothL1 box loss over bbox_preds  (B,A,4,H,W)
  - scalar means -> (total, cls_loss, box_loss)

Device strategy (one batch element per core), pixel-on-partition layout
(pixel = k*512 + j, k = partition), planes = (a, c) on the free axis.

Host-side (indexing / layout / dtype transforms only -- all arithmetic on
the big tensors stays on device):
  - inputs quantized to fp8-e4m3 (validated: rel err ~2e-4 vs the 2e-2 gate)
  - x_t = x[a, t(px), px] target-class logit plane gathered host-side
  - box loss touches only VALID pixels (~25% of 65536); host compacts the
    valid-pixel columns of bbox_preds/target boxes into a dense [128, 36, NVF]
    block (NaN-padded), shrinking both the DMA and the DVE stream 3-4x.
    Falls back to the full-pixel variant if n_valid ever exceeds the pad.

Per anchor PAIR, all chain ops on DVE (2x bf16) / ACT:
  e    = exp(x)               ACT   fp8 -> bf16, [128, 8, 512]
  s1   = e[::2] + e[1::2]     DVE
  S    = s1[::2] + s1[1::2]   DVE   (per-anchor softmax denominator)
  lnS  = ln(S)                ACT
  ce   = lnS - x_t            DVE
  pt   = exp(-ce)             ACT   (parallel with ace on DVE)
  ace  = alf * ce             DVE
  cls accum                   DVE   custom: sum (1-pt)^2 * ace
box (3 chunks of 3 anchors, compacted pixels):
  box accum                   DVE   custom: sum relu(|p-w|)^2 - relu(|p-w|-1)^2
                                    (w = target-box-or-NaN; relu(NaN)=0 on DVE
                                     scrubs invalid/pad pixels; == 2*smoothl1)
                                    w is a stride-0 broadcast AP over anchors.

host: final scalar reductions over the tiny per-partition accumulators.

The act-table pass is patched to use the single table set containing both
Exp and Ln (otherwise it thrashes between per-function sets).
"""

import sys

sys.path.insert(0, "/opt/trn_rl_repo")

from operator import add as _op_add

import ml_dtypes
import numpy as np

import concourse.bacc as bacc
import concourse.tile as tile
from concourse import mybir
from concourse.bass_utils import run_bass_kernel_spmd
from concourse.dve_spec import AluOp, Bin, C0, C1, One, Spec, Src0, Src1, lower, relu, sq
from concourse.dve_uop import DveOpSpec
import concourse.dve_ops as dvo

BF16 = mybir.dt.bfloat16
F8 = mybir.dt.float8e4
F32 = mybir.dt.float32
NP_F8 = ml_dtypes.float8_e4m3
NP_BF16 = ml_dtypes.bfloat16

GAMMA = 2.0
B, A, C, H, W, N = 8, 9, 4, 256, 256, 16
HW = H * W  # 65536
PARTS = 128
FREE = HW // PARTS  # 512
PAIRS = [(0, 2), (2, 4), (4, 6), (6, 8), (8, 9)]

NV_PAD = 24576  # padded valid-pixel count (actual ~10-17k; fallback beyond)
NVF = NV_PAD // PARTS  # 192 valid pixels per partition
BOX_CHUNKS = [(0, 3), (3, 6), (6, 9)]  # anchor ranges per SL1 call

# ---------------------------------------------------------------------------
# custom DVE ops
# ---------------------------------------------------------------------------


def _dve_relu(x):
    # DVE MAX semantics: max(NaN, 0) = 0 (numpy max propagates NaN)
    return np.maximum(np.nan_to_num(x, nan=0.0, posinf=np.inf, neginf=-np.inf), 0)


def _as_col(v, P):
    a = np.asarray(v, np.float32)
    return a.reshape(-1, 1) if a.ndim else np.full((P, 1), float(a), np.float32)


def _ref_sl1(in0, in1, s0, s1, imm2):
    P = in0.shape[0]
    a = np.abs(in0.astype(np.float32) - in1.astype(np.float32))
    body = _dve_relu(a) ** 2 - _dve_relu(a - _as_col(s0, P)) ** 2
    acc = _as_col(s1, P) + body.reshape(P, -1).sum(axis=-1, keepdims=True)
    return body.astype(np.float32), acc


def _ref_ft(in0, in1, s0, s1, imm2):
    P = in0.shape[0]
    body = (1.0 - in0.astype(np.float32)) ** 2 * in1.astype(np.float32)
    acc = _as_col(s0, P) + body.reshape(P, -1).sum(axis=-1, keepdims=True)
    return body.astype(np.float32), acc


def _register(name, spec):
    for op in dvo.OPS:
        if op.name == name:  # idempotent across re-imports
            return op
    op = dvo.DveOp(name, spec, subdim=False, uops_sha={})
    dvo.OPS.append(op)
    dvo.CUSTOM_DVE_SPECS[name] = spec
    dvo._SUB_OPCODE_FOR_NAME[name] = dvo._CUSTOM_DVE_ROW_BASE + len(dvo.OPS) - 1
    assert dvo._SUB_OPCODE_FOR_NAME[name] < 0x20
    for ver in ("v3", "v4"):
        sha = DveOpSpec(
            name=name,
            opcode=dvo.get_dve_sub_opcode(name),
            uops=lower(spec, ver=ver),
            rd1_en=True,
        ).sha(ver)
        op.uops_sha[ver] = sha
    return op


_absd = Bin(AluOp.ABSOLUTE_DIFF, Src0, Src1)
# accum_out[p] = s1 + sum_j relu(|in0-in1|)^2 - relu(|in0-in1| - s0)^2
# (AP seeding of the accumulator is broken -> literal 0.0, one column per call)
SL1_FUSED = _register(
    "SL1_FUSED_ANT",
    Spec(body=sq(relu(_absd)) - sq(relu(_absd - C0)), accum=_op_add,
         accum_init=C1, reference=_ref_sl1),
)
# accum_out[p] = s0 + sum_j (1 - in0)^2 * in1
FOCAL_TAIL = _register(
    "FOCAL_TAIL_ANT",
    Spec(body=sq(One - Src0) * Src1, accum=_op_add, accum_init=C0,
         reference=_ref_ft),
)

# ---------------------------------------------------------------------------
# device kernel (SPMD; one batch element per core)
# ---------------------------------------------------------------------------

_NC_CACHE = {}


def build_kernel(nvf):
    """nvf = valid pixels per partition in the compacted box block
    (NVF normally; FREE*A*C/(A) -- i.e. the full 512*36/36 -- on fallback)."""
    if nvf in _NC_CACHE:
        return _NC_CACHE[nvf]
    nc = bacc.Bacc()

    # pixel-on-partition packing: plane = a*C + c, free = j (512)
    cls_in = nc.dram_tensor("cls_in", [PARTS, A * C, FREE], F8, kind="ExternalInput")
    # compacted box block: inner 4*nvf = (c, j') per anchor
    boxc_in = nc.dram_tensor("boxc_in", [PARTS, A, C * nvf], F8, kind="ExternalInput")
    wnc_in = nc.dram_tensor("wnc_in", [PARTS, C * nvf], BF16, kind="ExternalInput")
    xt_in = nc.dram_tensor("xt_in", [PARTS, A, FREE], BF16, kind="ExternalInput")
    alf_in = nc.dram_tensor("alf_in", [PARTS, 2, FREE], BF16, kind="ExternalInput")
    out_cls = nc.dram_tensor("out_cls", [PARTS, len(PAIRS)], F32, kind="ExternalOutput")
    out_box = nc.dram_tensor(
        "out_box", [PARTS, len(BOX_CHUNKS)], F32, kind="ExternalOutput"
    )

    EXP = mybir.ActivationFunctionType.Exp
    LN = mybir.ActivationFunctionType.Ln

    with tile.TileContext(nc) as tc:
        with (
            tc.tile_pool(name="consts", bufs=1) as consts,
            tc.tile_pool(name="loads", bufs=2) as loads,
            tc.tile_pool(name="work", bufs=2) as work,
            tc.tile_pool(name="small", bufs=3) as small,
            tc.tile_pool(name="outs", bufs=1) as outs,
        ):
            acc_cls = outs.tile([PARTS, len(PAIRS)], F32)
            acc_box = outs.tile([PARTS, len(BOX_CHUNKS)], F32)

            # ---- compacted box loss (independent of the cls chain) ----
            boxc_t = consts.tile([PARTS, A, C * nvf], F8)
            nc.sync.dma_start(out=boxc_t, in_=boxc_in.ap())
            wnc_t = consts.tile([PARTS, C * nvf], BF16)
            nc.sync.dma_start(out=wnc_t, in_=wnc_in.ap())

            xt_t = consts.tile([PARTS, A, FREE], BF16)
            nc.sync.dma_start(out=xt_t, in_=xt_in.ap())
            alf_t = consts.tile([PARTS, 2, FREE], BF16)
            nc.sync.dma_start(out=alf_t, in_=alf_in.ap())

            for ci, (ca0, ca1) in enumerate(BOX_CHUNKS):
                na = ca1 - ca0
                sl_junk = work.tile([PARTS, 3, C * nvf], BF16, tag="sl_junk")
                nc.vector._custom_dve(
                    SL1_FUSED,
                    out=sl_junk[:, :na],
                    in0=boxc_t[:, ca0:ca1, :],
                    in1=wnc_t.unsqueeze(1).broadcast_to([PARTS, na, C * nvf]),
                    s0=1.0,
                    s1=0.0,
                    accum_out=acc_box[:, ci : ci + 1],
                )

            # ---- focal classification loss ----
            for pi, (a0, a1) in enumerate(PAIRS):
                na = a1 - a0
                nac = na * C
                x_t = loads.tile([PARTS, 2 * C, FREE], F8, tag="x_t")
                nc.sync.dma_start(
                    out=x_t[:, :nac], in_=cls_in.ap()[:, C * a0 : C * a1, :]
                )
                e_t = work.tile([PARTS, 2 * C, FREE], BF16, tag="e_t")
                nc.scalar.activation(e_t[:, :nac], x_t[:, :nac], EXP)

                # S = per-anchor sum over classes (pairwise tree)
                s1_t = small.tile([PARTS, C, FREE], BF16, tag="s1_t")
                nc.vector.tensor_add(
                    s1_t[:, : 2 * na], e_t[:, 0:nac:2, :], e_t[:, 1:nac:2, :]
                )
                s_t = small.tile([PARTS, 2, FREE], BF16, tag="s_t")
                nc.vector.tensor_add(
                    s_t[:, :na], s1_t[:, 0 : 2 * na : 2, :], s1_t[:, 1 : 2 * na : 2, :]
                )

                logs_t = small.tile([PARTS, 2, FREE], BF16, tag="logs_t")
                nc.scalar.activation(logs_t[:, :na], s_t[:, :na], LN)

                ce_t = small.tile([PARTS, 2, FREE], BF16, tag="ce_t")
                nc.vector.tensor_sub(ce_t[:, :na], logs_t[:, :na], xt_t[:, a0:a1, :])
                pt_t = small.tile([PARTS, 2, FREE], BF16, tag="pt_t")
                nc.scalar.activation(pt_t[:, :na], ce_t[:, :na], EXP, scale=-1.0)
                ace_t = small.tile([PARTS, 2, FREE], BF16, tag="ace_t")
                nc.vector.tensor_mul(ace_t[:, :na], alf_t[:, :na], ce_t[:, :na])

                ft_junk = small.tile([PARTS, 2, FREE], BF16, tag="ft_junk")
                nc.vector._custom_dve(
                    FOCAL_TAIL, out=ft_junk[:, :na], in0=pt_t[:, :na],
                    in1=ace_t[:, :na], s0=0.0, s1=0.0,
                    accum_out=acc_cls[:, pi : pi + 1],
                )

            nc.sync.dma_start(out=out_cls.ap(), in_=acc_cls)
            nc.sync.dma_start(out=out_box.ap(), in_=acc_box)

    # The act-table-load pass picks the FIRST set containing each function,
    # thrashing between exp_and_others and natural_log. Restrict eligibility
    # to the one set containing Exp AND Ln.
    _orig_gat = bacc.get_activation_tables
    _COMBINED = "natural_log_exp_and_others"

    def _patched_gat(arch):
        t = _orig_gat(arch)
        return {name: (fns if name == _COMBINED else set()) for name, fns in t.items()}

    bacc.get_activation_tables = _patched_gat
    try:
        nc.finalize()
    finally:
        bacc.get_activation_tables = _orig_gat
    _NC_CACHE[nvf] = nc
    return nc


# ---------------------------------------------------------------------------
# host side
# ---------------------------------------------------------------------------


def _rasterize_np(boxes, labels):
    """Exact numpy port of the reference _rasterize (truncation, clipping,
    last-covering-box-wins)."""
    Bn, Nn = labels.shape
    bi = boxes.astype(np.int32)
    x1 = np.clip(bi[..., 0], 0, W - 1)
    y1 = np.clip(bi[..., 1], 0, H - 1)
    x2 = np.clip(bi[..., 2], 0, W - 1)
    y2 = np.clip(bi[..., 3], 0, H - 1)
    ys = np.arange(H)
    xs = np.arange(W)
    inside = (
        (ys[None, None, :, None] >= y1[:, :, None, None])
        & (ys[None, None, :, None] <= y2[:, :, None, None])
        & (xs[None, None, None, :] >= x1[:, :, None, None])
        & (xs[None, None, None, :] <= x2[:, :, None, None])
    )  # (B,N,H,W)
    box_ids = np.arange(Nn, dtype=np.int32)[None, :, None, None]
    last = np.max(np.where(inside, box_ids, -1), axis=1)  # (B,H,W)
    valid = last >= 0
    idx = np.maximum(last, 0)
    bsel = np.arange(Bn)[:, None, None]
    tgt_label = np.where(valid, labels[bsel, idx], 0)  # (B,H,W)
    tgt_box = boxes[bsel, idx]  # (B,H,W,4)
    return tgt_label, tgt_box, valid


_LAST_RESULT = None  # BassKernelResults of the last run (for profiling)


def kernel(cls_scores, bbox_preds, boxes, labels, alpha):
    global _LAST_RESULT
    cls_scores = np.ascontiguousarray(cls_scores, dtype=np.float32)
    bbox_preds = np.ascontiguousarray(bbox_preds, dtype=np.float32)
    boxes = np.asarray(boxes, dtype=np.float32)
    labels = np.asarray(labels, dtype=np.int32)
    alpha = np.asarray(alpha, dtype=np.float32)

    tgt_label, tgt_box, valid = _rasterize_np(boxes, labels)

    qx = cls_scores.astype(NP_F8)  # (B,A,C,H,W) quantized logits
    qp = bbox_preds.astype(NP_F8)

    nv_max = int(valid.reshape(B, -1).sum(axis=1).max())
    nvf = NVF if nv_max <= NV_PAD else FREE  # fallback: all pixels, no compact

    in_maps = []
    for b in range(B):
        t = tgt_label[b].reshape(HW)  # int, [65536]
        v = valid[b].reshape(HW)
        # x_t gathered from the QUANTIZED logits (bit-consistent with cls_in)
        xb = qx[b].reshape(A, C, HW)
        xt = np.take_along_axis(
            xb.view(np.uint8), np.broadcast_to(t[None, None, :], (A, 1, HW)), axis=1
        )[:, 0].view(NP_F8)  # (A, HW)
        xt = xt.reshape(A, PARTS, FREE).transpose(1, 0, 2)  # [128, 9, 512]
        alf = alpha[t].reshape(PARTS, FREE)
        alf2 = np.broadcast_to(alf[:, None, :], (PARTS, 2, FREE))
        # compacted box block: valid pixel columns, NaN-padded to PARTS*nvf
        nvp = PARTS * nvf
        if nvf == FREE:
            vidx = np.arange(HW)
        else:
            vidx = np.flatnonzero(v)
        npad = nvp - len(vidx)
        pb = qp[b].reshape(A, C, HW)[:, :, vidx]  # (A, C, nv)
        pb = np.concatenate(
            [pb, np.zeros((A, C, npad), NP_F8)], axis=2
        )  # (A, C, nvp)
        # layout [128, A, C*nvf]: pixel j' = k*nvf + f
        pb = (
            pb.reshape(A, C, PARTS, nvf).transpose(2, 0, 1, 3).reshape(PARTS, A, C * nvf)
        )
        tb = tgt_box[b].reshape(HW, 4).T  # [4, 65536]
        wv = np.where(v[None, :], tb, np.nan)[:, vidx].astype(np.float32)  # (4, nv)
        wv = np.concatenate([wv, np.full((C, npad), np.nan, np.float32)], axis=1)
        wv = wv.reshape(C, PARTS, nvf).transpose(1, 0, 2).reshape(PARTS, C * nvf)
        in_maps.append(
            {
                "cls_in": np.ascontiguousarray(
                    qx[b].reshape(A * C, PARTS, FREE).transpose(1, 0, 2)
                ),
                "boxc_in": np.ascontiguousarray(pb),
                "wnc_in": np.ascontiguousarray(wv.astype(NP_BF16)),
                "xt_in": np.ascontiguousarray(xt.astype(NP_BF16)),
                "alf_in": np.ascontiguousarray(alf2.astype(NP_BF16)),
            }
        )

    nc = build_kernel(nvf)
    res = run_bass_kernel_spmd(nc, in_maps, core_ids=list(range(B)))
    _LAST_RESULT = res

    cls_loss_b = np.empty(B, np.float64)
    box_loss_b = np.empty(B, np.float64)
    for b in range(B):
        cls_sum = float(res.results[b]["out_cls"].astype(np.float64).sum())
        box_sum = float(res.results[b]["out_box"].astype(np.float64).sum()) * 0.5
        cls_loss_b[b] = cls_sum / (A * HW)
        cnt = float(valid[b].sum()) * (A * 4)
        box_loss_b[b] = box_sum / max(cnt, 1.0) if cnt > 0 else 0.0

    cls_loss = np.float32(cls_loss_b.mean())
    box_loss = np.float32(box_loss_b.mean())
    total = np.float32(cls_loss + box_loss)
    return total, cls_loss, box_loss
